# revision 13
# baseline (speedup 1.0000x reference)
"""Local-strided block-sparse paged attention (decode) on 8 Trainium2 cores.

Strategy (memory-regime): the 4 q-heads of a GQA group share one kv head, so
their CSR rows overlap heavily (the local window is common; only the stride
phase differs).  Host resolves, per (sequence, kv-head) unit, the UNION of the
4 heads' attended kv blocks and gathers K/V once for the union instead of 4x
per head -- ~2x fewer bytes.  Panels ship as bf16 (2x fewer bytes again) and
per-head masking happens on-chip.  Units are bucketed into 8 size-sorted slots
(one unit per slot per core) so a single SPMD program with per-slot static
shapes stays load-balanced and near the exact-union byte count.

Device per unit: QK^T via per-chunk matmuls (K chunk stationary, 4 q columns
moving) -> +mask on DVE -> exp on ACT -> PV via accumulating matmuls with a
ones-column appended to V so the softmax denominator falls out of the same
matmul -> reciprocal + scale on DVE.  All FP work and all K/V HBM traffic is
on-device; the host only does int index resolution (control plane) and the
gather/layout, mirroring how a serving framework prepares block tables.
"""
import numpy as np

B, H, KVH, D, X = 16, 16, 4, 128, 4
BLK, MAXB = 16, 256
NC_CORES = 8
NSLOT = 8                      # units per core = 64 units / 8 cores
SM_SCALE = 1.0 / float(np.sqrt(D))
NEG = np.float32(-1e9)


def _np_dt(name):
    import concourse.mybir as mybir
    return mybir.dt.np(getattr(mybir.dt, name))


_PROGRAMS = {}


def _build_program(sizes, repeat=1, no_compute=False, no_dma=False, empty=False):
    """sizes: descending tuple of per-slot union block counts (multiples of 8).
    repeat>1 wraps the body in a device-side loop (for timing: one dispatch
    runs the kernel `repeat` times, so the ~2ms axon dispatch RTT amortizes).
    no_compute/no_dma/empty build crippled variants for engine attribution."""
    import contextlib
    import concourse.bacc as bacc
    import concourse.mybir as mybir
    from concourse.tile import TileContext

    f32 = mybir.dt.float32
    bf16 = mybir.dt.bfloat16
    NCHs = [s * 16 // 128 for s in sizes]
    sumP = sum(s * 16 + n * 129 for s, n in zip(sizes, NCHs))
    sumM = sum(n * 4 for n in NCHs)

    nc = bacc.Bacc("TRN2", target_bir_lowering=False)
    pd = nc.dram_tensor("pd", [128, sumP], bf16, kind="ExternalInput")
    md = nc.dram_tensor("md", [128, sumM], f32, kind="ExternalInput")
    qd = nc.dram_tensor("qd", [128, 4 * NSLOT], bf16, kind="ExternalInput")
    out = nc.dram_tensor("out", [4, 128 * NSLOT], f32, kind="ExternalOutput")

    with TileContext(nc) as tc:
        with (
            tc.tile_pool(name="kv", bufs=3) as kvp,
            tc.tile_pool(name="sp", bufs=3) as sp,
            tc.tile_pool(name="cst", bufs=1) as cp,
            tc.tile_pool(name="ps_sc", bufs=2, space="PSUM") as pp_sc,
            tc.tile_pool(name="ps_ov", bufs=2, space="PSUM") as pp_ov,
        ):
            if repeat > 1:
                # pull the one-time ACT exp-table load out of the timed loop
                warm = cp.tile([1, 1], f32, tag="warm")
                nc.vector.memset(warm[:], 0.0)
                nc.scalar.activation(
                    warm[:], warm[:], mybir.ActivationFunctionType.Exp,
                )
            rep_ctx = (
                tc.For_i(0, repeat, 1, hint_engines=(mybir.EngineType.PE,))
                if repeat > 1 else contextlib.nullcontext()
            )
            with rep_ctx:
                qt = cp.tile([128, 4 * NSLOT], bf16, tag="qt")
                if empty:
                    nc.vector.memset(qt[:], 0.0)
                else:
                    nc.sync.dma_start(out=qt[:], in_=qd[:, :])
                mt = cp.tile([128, sumM], f32, tag="mt")
                if not empty:
                    nc.sync.dma_start(out=mt[:], in_=md[:, :])
                if not (empty or no_compute):
                    osb = cp.tile([4, 128 * NSLOT], f32, tag="osb")

                po = mo = 0
                for r in range(NSLOT if not empty else 0):
                    T = sizes[r] * 16
                    NCH = NCHs[r]
                    W = T + NCH * 129
                    pan = kvp.tile([128, W], bf16, tag="pan")
                    if not no_dma:
                        nc.sync.dma_start(out=pan[:], in_=pd[:, po:po + W])
                    kt = pan[:, 0:T]
                    vt = pan[:, T:W]
                    if no_compute:
                        po += W
                        mo += NCH * 4
                        continue

                    sc = pp_sc.tile([128, NCH * 4], f32, tag="sc")
                    for c in range(NCH):
                        nc.tensor.matmul(
                            sc[:, 4 * c:4 * c + 4],
                            kt[:, 128 * c:128 * (c + 1)],
                            qt[:, 4 * r:4 * r + 4],
                            start=True, stop=True,
                        )
                    ssb = sp.tile([128, NCH * 4], f32, tag="ssb")
                    nc.vector.tensor_add(ssb[:], sc[:], mt[:, mo:mo + NCH * 4])
                    p = sp.tile([128, NCH * 4], bf16, tag="p")
                    nc.scalar.activation(
                        p[:], ssb[:], mybir.ActivationFunctionType.Exp, scale=SM_SCALE,
                    )
                    ov = pp_ov.tile([4, 129], f32, tag="ov")
                    for c in range(NCH):
                        nc.tensor.matmul(
                            ov[:],
                            p[:, 4 * c:4 * c + 4],
                            vt[:, 129 * c:129 * (c + 1)],
                            start=(c == 0), stop=(c == NCH - 1),
                        )
                    rec = sp.tile([4, 1], f32, tag="rec")
                    nc.vector.reciprocal(rec[:], ov[:, 128:129])
                    nc.vector.tensor_scalar_mul(
                        osb[:, 128 * r:128 * (r + 1)], ov[:, 0:128], rec[:],
                    )
                    po += W
                    mo += NCH * 4
                if not (empty or no_compute):
                    nc.sync.dma_start(out=out[:, :], in_=osb[:])
    nc.compile()
    return nc


def _get_program(sizes):
    key = tuple(sizes)
    if key not in _PROGRAMS:
        _PROGRAMS[key] = _build_program(key)
    return _PROGRAMS[key]


def _prepare(q, k_cache, v_cache, block_tables, context_lens, layout_crow, layout_col):
    """Resolve CSR -> per-unit union panels; bucket units into slots; build
    per-core input maps.  Returns (sizes, in_maps, assign) where
    assign[(core, slot)] = (b, g)."""
    BF16 = _np_dt("bfloat16")

    q_pid = context_lens.astype(np.int64) - 1
    pbid = q_pid // BLK

    units = []  # (b, g, u, memb)
    for b in range(B):
        kept = []
        for h in range(H):
            s, e = int(layout_crow[h, pbid[b]]), int(layout_crow[h, pbid[b] + 1])
            kept.append(np.asarray(layout_col[h, s:e]))
        for g in range(KVH):
            hs = kept[4 * g:4 * g + 4]
            u = np.unique(np.concatenate(hs))
            memb = np.stack([np.isin(u, kh) for kh in hs], axis=1)  # [L,4]
            units.append((b, g, u, memb))

    L_all = np.array([len(t[2]) for t in units])
    order = np.argsort(-L_all, kind="stable")
    sizes = []
    assign = {}
    for r in range(NSLOT):
        grp = order[r * NC_CORES:(r + 1) * NC_CORES]
        S = max(8, int(-(-int(L_all[grp].max()) // 8) * 8))
        sizes.append(S)
        for c, ui in enumerate(grp):
            assign[(c, r)] = int(ui)
    NCHs = [s * 16 // 128 for s in sizes]
    sumP = sum(s * 16 + n * 129 for s, n in zip(sizes, NCHs))
    sumM = sum(n * 4 for n in NCHs)

    in_maps = []
    for c in range(NC_CORES):
        pdv = np.zeros((128, sumP), BF16)
        mdv = np.full((128, sumM), NEG, np.float32)
        qdv = np.zeros((128, 4 * NSLOT), BF16)
        po = mo = 0
        for r in range(NSLOT):
            b, g, u, memb = units[assign[(c, r)]]
            L = len(u)
            T = sizes[r] * 16
            NCH = NCHs[r]
            phys = block_tables[b, u]
            kb = k_cache[phys, g]                                  # [L,32,16,4]
            kpan = kb.transpose(1, 3, 0, 2).reshape(128, L * 16)   # [d,t]
            pdv[:, po:po + L * 16] = kpan.astype(BF16)
            vb = v_cache[phys, g]                                  # [L,128,16]
            vtp = np.zeros((T, 128), np.float32)
            vtp[:L * 16] = vb.transpose(0, 2, 1).reshape(L * 16, 128)
            vfull = np.concatenate(
                [vtp.reshape(NCH, 128, 128), np.ones((NCH, 128, 1), np.float32)],
                axis=2,
            )                                                      # [NCH,128,129]
            pdv[:, po + T:po + T + NCH * 129] = (
                vfull.transpose(1, 0, 2).reshape(128, NCH * 129).astype(BF16)
            )
            pos = u[:, None].astype(np.int64) * BLK + np.arange(BLK)
            tv = memb[:, None, :] & (pos <= q_pid[b])[:, :, None]  # [L,16,4]
            mv = np.where(tv.reshape(L * 16, 4), np.float32(0.0), NEG)
            mp = np.full((T, 4), NEG, np.float32)
            mp[:L * 16] = mv
            mdv[:, mo:mo + NCH * 4] = mp.reshape(NCH, 128, 4).transpose(1, 0, 2).reshape(128, NCH * 4)
            qdv[:, 4 * r:4 * r + 4] = q[b, 4 * g:4 * g + 4].T.astype(BF16)
            po += T + NCH * 129
            mo += NCH * 4
        in_maps.append({"pd": pdv, "md": mdv, "qd": qdv})
    return tuple(sizes), in_maps, assign


def kernel(q, k_cache, v_cache, block_tables, context_lens, layout_crow, layout_col):
    from concourse.bass_utils import run_bass_kernel_spmd

    q = np.asarray(q, np.float32)
    k_cache = np.asarray(k_cache, np.float32)
    v_cache = np.asarray(v_cache, np.float32)
    block_tables = np.asarray(block_tables, np.int32)
    context_lens = np.asarray(context_lens, np.int32)
    layout_crow = np.asarray(layout_crow, np.int32)
    layout_col = np.asarray(layout_col, np.int32)

    sizes, in_maps, assign = _prepare(
        q, k_cache, v_cache, block_tables, context_lens, layout_crow, layout_col
    )
    nc = _get_program(sizes)
    res = run_bass_kernel_spmd(nc, in_maps, core_ids=list(range(NC_CORES)))
    out = np.zeros((B, H, D), np.float32)
    for (c, r), ui in assign.items():
        b = ui // KVH
        g = ui % KVH
        out[b, 4 * g:4 * g + 4] = res.results[c]["out"][:, 128 * r:128 * (r + 1)]
    return out


# revision 29
# speedup vs baseline: 1.4829x; 1.4829x over previous
"""Local-strided block-sparse paged attention (decode) on 8 Trainium2 cores.

Strategy (memory-regime): the 4 q-heads of a GQA group share one kv head, so
their CSR rows overlap heavily (the local window is common; only the stride
phase differs).  Host resolves, per (sequence, kv-head) unit, the UNION of the
4 heads' attended kv blocks and gathers K/V once for the union instead of 4x
per head -- ~2x fewer bytes.  Panels ship as bf16 (2x fewer bytes again) with
the per-head additive masks folded in; masking happens on-chip.  Units are
bucketed into 8 size-sorted slots (one unit per slot per core) so a single
SPMD program with per-slot static shapes stays load-balanced and close to the
exact-union byte count.

Device per unit: QK^T via per-chunk matmuls (K chunk stationary bf16+FWL,
4 q columns moving) -> +mask on DVE -> exp on ACT -> PV via accumulating
matmuls with a ones-column appended to V so the softmax denominator falls out
of the same matmul -> reciprocal + scale on DVE.  All FP work and all K/V HBM
traffic is on-device; the host only does int index resolution (control plane)
and the gather/layout, mirroring how a serving framework prepares block
tables.  Slot 0's panel is fetched in two DMAs (K first) so the PE starts
~1.5us earlier; later slots stream as one ~1MB DMA each to stay near peak
HBM bandwidth.
"""
import numpy as np

B, H, KVH, D, X = 16, 16, 4, 128, 4
BLK, MAXB = 16, 256
NC_CORES = 8
NSLOT = 8                      # units per core = 64 units / 8 cores
SM_SCALE = 1.0 / float(np.sqrt(D))
NEG = np.float32(-1e9)


def _np_dt(name):
    import concourse.mybir as mybir
    return mybir.dt.np(getattr(mybir.dt, name))


_PROGRAMS = {}


def _build_program(sizes, repeat=1, no_compute=False, no_dma=False, empty=False,
                   split_first=True, split_last=True, split_out=True,
                   alt_engine=True, kv_bufs=6, sc_bufs=4):
    """sizes: descending tuple of per-slot union block counts (multiples of 8).
    repeat>1 wraps the body in a device-side loop (for timing: one dispatch
    runs the kernel `repeat` times, amortizing the ~2ms axon dispatch RTT).
    no_compute/no_dma/empty build crippled variants for engine attribution."""
    import contextlib
    import concourse.bacc as bacc
    import concourse.mybir as mybir
    from concourse.tile import TileContext

    f32 = mybir.dt.float32
    bf16 = mybir.dt.bfloat16
    NCHs = [s * 16 // 128 for s in sizes]
    Ws = [s * 16 + n * 133 for s, n in zip(sizes, NCHs)]  # K | V+ones | mask
    p_off = [sum(Ws[:r]) for r in range(NSLOT)]
    sumP = sum(Ws)

    nc = bacc.Bacc("TRN2", target_bir_lowering=False)
    pd = nc.dram_tensor("pd", [128, sumP], bf16, kind="ExternalInput")
    qd = nc.dram_tensor("qd", [128, 4 * NSLOT], bf16, kind="ExternalInput")
    out = nc.dram_tensor("out", [4, 128 * NSLOT], f32, kind="ExternalOutput")

    with TileContext(nc) as tc:
        with (
            tc.tile_pool(name="kv", bufs=kv_bufs) as kvp,
            tc.tile_pool(name="sp", bufs=3) as sp,
            tc.tile_pool(name="cst", bufs=1) as cp,
            tc.tile_pool(name="ps_sc", bufs=sc_bufs, space="PSUM") as pp_sc,
            tc.tile_pool(name="ps_ov", bufs=2, space="PSUM") as pp_ov,
        ):
            if repeat > 1:
                # pull the one-time ACT exp-table load out of the timed loop
                warm = cp.tile([1, 1], f32, tag="warm")
                nc.vector.memset(warm[:], 0.0)
                nc.scalar.activation(
                    warm[:], warm[:], mybir.ActivationFunctionType.Exp,
                )
            rep_ctx = (
                tc.For_i(0, repeat, 1, hint_engines=(mybir.EngineType.PE,))
                if repeat > 1 else contextlib.nullcontext()
            )
            with rep_ctx:
                qt = cp.tile([128, 4 * NSLOT], bf16, tag="qt")
                if empty:
                    nc.vector.memset(qt[:], 0.0)
                else:
                    nc.sync.dma_start(out=qt[:], in_=qd[:, :])
                if not (empty or no_compute):
                    osb = cp.tile([4, 128 * NSLOT], f32, tag="osb")

                def emit_slot(r, kt, vt, mt):
                    NCH = NCHs[r]
                    sc = pp_sc.tile([128, NCH * 4], f32, tag="sc")
                    for c in range(NCH):
                        nc.tensor.matmul(
                            sc[:, 4 * c:4 * c + 4],
                            kt[:, 128 * c:128 * (c + 1)],
                            qt[:, 4 * r:4 * r + 4],
                            start=True, stop=True,
                        )
                    ssb = sp.tile([128, NCH * 4], f32, tag="ssb")
                    nc.vector.tensor_add(ssb[:], sc[:], mt[:])
                    p = sp.tile([128, NCH * 4], bf16, tag="p")
                    nc.scalar.activation(
                        p[:], ssb[:], mybir.ActivationFunctionType.Exp,
                        scale=SM_SCALE,
                    )
                    ov = pp_ov.tile([4, 129], f32, tag="ov")
                    for c in range(NCH):
                        nc.tensor.matmul(
                            ov[:],
                            p[:, 4 * c:4 * c + 4],
                            vt[:, 129 * c:129 * (c + 1)],
                            start=(c == 0), stop=(c == NCH - 1),
                        )
                    rec = sp.tile([4, 1], f32, tag="rec")
                    nc.vector.reciprocal(rec[:], ov[:, 128:129])
                    nc.vector.tensor_scalar_mul(
                        osb[:, 128 * r:128 * (r + 1)], ov[:, 0:128], rec[:],
                    )

                for r in range(NSLOT if not empty else 0):
                    T = sizes[r] * 16
                    NCH = NCHs[r]
                    W = Ws[r]
                    po = p_off[r]
                    eng = nc.scalar if (alt_engine and r % 2) else nc.sync
                    pan = kvp.tile([128, W], bf16, tag="kt")
                    if no_dma:
                        # token write so Tile sees the tile as allocated
                        eng.dma_start(out=pan[:, 0:1], in_=pd[:, po:po + 1])
                    elif (r == 0 and split_first) or (r == NSLOT - 1 and split_last):
                        eng.dma_start(out=pan[:, 0:T], in_=pd[:, po:po + T])
                        eng.dma_start(out=pan[:, T:W], in_=pd[:, po + T:po + W])
                    else:
                        eng.dma_start(out=pan[:], in_=pd[:, po:po + W])
                    if not no_compute:
                        emit_slot(
                            r,
                            pan[:, 0:T],
                            pan[:, T:T + NCH * 129],
                            pan[:, T + NCH * 129:W],
                        )
                        if split_out and r == NSLOT - 2:
                            nc.sync.dma_start(
                                out=out[:, 0:128 * (NSLOT - 1)],
                                in_=osb[:, 0:128 * (NSLOT - 1)],
                            )
                if not (empty or no_compute):
                    if split_out:
                        nc.sync.dma_start(
                            out=out[:, 128 * (NSLOT - 1):],
                            in_=osb[:, 128 * (NSLOT - 1):],
                        )
                    else:
                        nc.sync.dma_start(out=out[:, :], in_=osb[:])
    nc.compile()
    return nc


def _get_program(sizes):
    key = tuple(sizes)
    if key not in _PROGRAMS:
        _PROGRAMS[key] = _build_program(key)
    return _PROGRAMS[key]


def _prepare(q, k_cache, v_cache, block_tables, context_lens, layout_crow, layout_col):
    """Resolve CSR -> per-unit union panels; bucket units into slots; build
    per-core input maps.  Returns (sizes, in_maps, assign) where
    assign[(core, slot)] = unit index (b * KVH + g)."""
    BF16 = _np_dt("bfloat16")

    q_pid = context_lens.astype(np.int64) - 1
    pbid = q_pid // BLK

    units = []  # (b, g, u, memb)
    for b in range(B):
        kept = []
        for h in range(H):
            s, e = int(layout_crow[h, pbid[b]]), int(layout_crow[h, pbid[b] + 1])
            kept.append(np.asarray(layout_col[h, s:e]))
        for g in range(KVH):
            hs = kept[4 * g:4 * g + 4]
            u = np.unique(np.concatenate(hs))
            memb = np.stack([np.isin(u, kh) for kh in hs], axis=1)  # [L,4]
            units.append((b, g, u, memb))

    L_all = np.array([len(t[2]) for t in units])
    order = np.argsort(-L_all, kind="stable")
    sizes = []
    assign = {}
    for r in range(NSLOT):
        grp = order[r * NC_CORES:(r + 1) * NC_CORES]
        S = max(8, int(-(-int(L_all[grp].max()) // 8) * 8))
        sizes.append(S)
        for c, ui in enumerate(grp):
            assign[(c, r)] = int(ui)
    NCHs = [s * 16 // 128 for s in sizes]
    Ws = [s * 16 + n * 133 for s, n in zip(sizes, NCHs)]
    sumP = sum(Ws)

    in_maps = []
    for c in range(NC_CORES):
        pdv = np.zeros((128, sumP), BF16)
        qdv = np.zeros((128, 4 * NSLOT), BF16)
        po = 0
        for r in range(NSLOT):
            b, g, u, memb = units[assign[(c, r)]]
            L = len(u)
            T = sizes[r] * 16
            NCH = NCHs[r]
            phys = block_tables[b, u]
            kb = k_cache[phys, g]                                  # [L,32,16,4]
            kpan = kb.transpose(1, 3, 0, 2).reshape(128, L * 16)   # [d,t]
            pdv[:, po:po + L * 16] = kpan.astype(BF16)
            vb = v_cache[phys, g]                                  # [L,128,16]
            vtp = np.zeros((T, 128), np.float32)
            vtp[:L * 16] = vb.transpose(0, 2, 1).reshape(L * 16, 128)
            vfull = np.concatenate(
                [vtp.reshape(NCH, 128, 128), np.ones((NCH, 128, 1), np.float32)],
                axis=2,
            )                                                      # [NCH,128,129]
            pdv[:, po + T:po + T + NCH * 129] = (
                vfull.transpose(1, 0, 2).reshape(128, NCH * 129).astype(BF16)
            )
            pos = u[:, None].astype(np.int64) * BLK + np.arange(BLK)
            tv = memb[:, None, :] & (pos <= q_pid[b])[:, :, None]  # [L,16,4]
            mv = np.where(tv.reshape(L * 16, 4), np.float32(0.0), NEG)
            mp = np.full((T, 4), NEG, np.float32)
            mp[:L * 16] = mv
            pdv[:, po + T + NCH * 129:po + Ws[r]] = (
                mp.reshape(NCH, 128, 4).transpose(1, 0, 2)
                .reshape(128, NCH * 4).astype(BF16)
            )
            qdv[:, 4 * r:4 * r + 4] = q[b, 4 * g:4 * g + 4].T.astype(BF16)
            po += Ws[r]
        in_maps.append({"pd": pdv, "qd": qdv})
    return tuple(sizes), in_maps, assign


def kernel(q, k_cache, v_cache, block_tables, context_lens, layout_crow, layout_col):
    from concourse.bass_utils import run_bass_kernel_spmd

    q = np.asarray(q, np.float32)
    k_cache = np.asarray(k_cache, np.float32)
    v_cache = np.asarray(v_cache, np.float32)
    block_tables = np.asarray(block_tables, np.int32)
    context_lens = np.asarray(context_lens, np.int32)
    layout_crow = np.asarray(layout_crow, np.int32)
    layout_col = np.asarray(layout_col, np.int32)

    sizes, in_maps, assign = _prepare(
        q, k_cache, v_cache, block_tables, context_lens, layout_crow, layout_col
    )
    nc = _get_program(sizes)
    res = run_bass_kernel_spmd(nc, in_maps, core_ids=list(range(NC_CORES)))
    out = np.zeros((B, H, D), np.float32)
    for (c, r), ui in assign.items():
        b = ui // KVH
        g = ui % KVH
        out[b, 4 * g:4 * g + 4] = res.results[c]["out"][:, 128 * r:128 * (r + 1)]
    return out
